# revision 21
# baseline (speedup 1.0000x reference)
"""GCAT (graph attention over ent/rel/attr embeddings) on 8 Trainium2 cores.

Sharding: edges are grouped 16-per-node and node-sorted, so we shard nodes
(and thus edges) into 8 contiguous blocks, one per core.  Embedding tables
and attention kernels are replicated.  Per layer each core computes its
nodes' features, all-gathers the [N,128] feature table (+ the [N,4]
per-node neighbor score table), then does per-edge gathers from the
replicated tables with dma_gather.

dma_gather indices are int16 (signed).  Rows >= 32768 are addressed via a
sign-wraparound trick: the gather base points at row 32768 of a region
whose first 17232 rows duplicate table rows 32768..49999, so negative
int16 indices (col - 65536) land on the duplicate copy.
"""

import sys

sys.path.insert(0, "/opt/trn_rl_repo")

import numpy as np

import concourse.bacc as bacc
import concourse.bass as bass
import concourse.bass_isa as bass_isa
import concourse.tile as tile
from concourse import mybir
from concourse.bass_utils import run_bass_kernel_spmd
from concourse.tile_rust import add_dep_helper

F32 = mybir.dt.float32
I16 = mybir.dt.int16
AF = mybir.ActivationFunctionType
OP = mybir.AluOpType

NCORES = 8
N = 50000
DEG = 16
E = N * DEG
R = 1000
A = 2000
ADEG = 4
DOUT = 128
H = 4
DH = 32
L = 2
NB = N // NCORES          # 6250 nodes per core
EB = NB * DEG             # 100000 edges per core
TP = 128                  # nodes per tile
TILES = (NB + TP - 1) // TP   # 49
NPAD = TILES * TP         # 6272
LASTP = NB - (TILES - 1) * TP  # 106 valid partitions in the last tile
HALF = 32768
EXTRA = N - HALF          # 17232 duplicated rows
RX = HALF + N             # 82768 region rows
NI = TP * DEG             # 2048 gather indices per tile
NI2 = NI + 16             # +16 zero-index pad so the stream never ends negative
                          # (the Q7 firmware drops a trailing run of negative
                          # int16 indices; mid-stream negatives are processed)
NIA = TP * ADEG           # 512 attr indices per tile
NEG_BIG = -1.0e9

_cached = {}


def _ap(t, width, offset, pairs, nparts=128, poff=0):
    """Strided view of a [nparts, width] SBUF tile. pairs = [[step, count], ...]"""
    base = t[:]
    return bass.AP(
        tensor=base.tensor,
        offset=base.offset + poff * width + offset,
        ap=[[width, nparts]] + [list(p) for p in pairs],
    )


def _build(debug=False):
    nc = bacc.Bacc(num_devices=NCORES)
    dbg = None
    if debug:
        dbg = nc.dram_tensor("dbg", [128, 704], F32, kind="ExternalOutput")

    ent_region = nc.dram_tensor("ent_region", [RX, DOUT], F32, kind="ExternalInput")
    rel_tab = nc.dram_tensor("rel_tab", [R, 64], F32, kind="ExternalInput")
    attr_tab = nc.dram_tensor("attr_tab", [A, 64], F32, kind="ExternalInput")
    kvecs = nc.dram_tensor("kvecs", [1, L * 6 * 128], F32, kind="ExternalInput")
    cidx = nc.dram_tensor("cidx", [TILES, 128, NI2 // 16], I16, kind="ExternalInput")
    ridx = nc.dram_tensor("ridx", [TILES, 128, NI // 16], I16, kind="ExternalInput")
    aidx = nc.dram_tensor("aidx", [TILES, 128, NIA // 16], I16, kind="ExternalInput")
    padmask = nc.dram_tensor("padmask", [128, 1], F32, kind="ExternalInput")
    out = nc.dram_tensor("out", [NB, L * DOUT], F32, kind="ExternalOutput")

    # Internal DRAM (per layer to avoid cross-layer races through collectives)
    region_x = [nc.dram_tensor(f"region_x{i}", [RX, DOUT], F32, kind="Internal",
                               addr_space="Shared") for i in range(L)]
    region_n = [nc.dram_tensor(f"region_n{i}", [RX, 64], F32, kind="Internal")
                for i in range(L)]
    n_full = [nc.dram_tensor(f"n_full{i}", [N, 4], F32, kind="Internal",
                             addr_space="Shared") for i in range(L)]
    stats_full = [nc.dram_tensor(f"stats_full{i}", [NCORES, 8], F32, kind="Internal",
                                 addr_space="Shared") for i in range(L)]
    agx_in = nc.dram_tensor("agx_in", [NB, DOUT], F32, kind="Internal")
    agn_in = nc.dram_tensor("agn_in", [NB, 4], F32, kind="Internal")
    stats_in = nc.dram_tensor("stats_in", [1, 8], F32, kind="Internal")

    RG = [list(range(NCORES))]

    with tile.TileContext(nc) as tc:
        with (
            tc.tile_pool(name="per", bufs=1) as per,          # persistent buffers
            tc.tile_pool(name="gbig", bufs=2) as gbig,        # 2048-col gather tiles
            tc.tile_pool(name="gsm", bufs=2) as gsm,          # 1024-col gather tiles
            tc.tile_pool(name="wgp", bufs=2) as wgp,          # weighted/tree scratch
            tc.tile_pool(name="idxp", bufs=2) as idxp,        # idx tiles
            tc.tile_pool(name="smp", bufs=2) as smp,          # small scratch
        ):
            W_E = TILES * 128      # 6272
            W_S = TILES * 64       # 3136
            W_C = TILES * 32       # 1568
            W_4 = TILES * 4        # 196

            e_all = per.tile([128, W_E], F32, tag="e_all")
            d_all = per.tile([128, W_E], F32, tag="d_all")
            srel_all = per.tile([128, W_E], F32, tag="srel_all")
            crel_all = per.tile([128, W_C], F32, tag="crel_all")
            cattr_all = per.tile([128, W_C], F32, tag="cattr_all")
            s_all = per.tile([128, W_S], F32, tag="s_all")
            z_all = per.tile([128, W_S], F32, tag="z_all")   # also: n_all, leaky scratch
            a_all = per.tile([128, W_S], F32, tag="a_all")
            self_all = per.tile([128, W_4], F32, tag="self_all")
            neigh_all = per.tile([128, W_4], F32, tag="neigh_all")
            t196 = per.tile([128, W_4], F32, tag="t196")
            mrow = per.tile([128, W_4], F32, tag="mrow")
            den = per.tile([128, W_4], F32, tag="den")
            kv_b = per.tile([128, 768], F32, tag="kv_b")
            m4a = per.tile([128, 4], F32, tag="m4a")   # mloc
            m4b = per.tile([128, 4], F32, tag="m4b")   # ssum
            m4c = per.tile([128, 4], F32, tag="m4c")   # B
            m4d = per.tile([128, 4], F32, tag="m4d")   # c = exp(mloc - B)
            m4e = per.tile([128, 4], F32, tag="m4e")   # scratch
            m4a2 = per.tile([128, 4], F32, tag="m4a2")
            m4b2 = per.tile([128, 4], F32, tag="m4b2")
            sb8c = per.tile([8, 4], F32, tag="sb8c")
            st8 = per.tile([8, 8], F32, tag="st8")
            sb8a = per.tile([8, 4], F32, tag="sb8a")
            sb8b = per.tile([8, 4], F32, tag="sb8b")
            stats_sb = per.tile([1, 8], F32, tag="stats_sb")
            pm = per.tile([128, 1], F32, tag="pm")
            nc.sync.dma_start(pm[:], padmask[:])

            def load_idx(src, t, cols):
                it = idxp.tile([128, 132], I16, tag="idx")
                nc.sync.dma_start(it[:, :cols], src[t, :, :])
                return it

            # ---------------- phase 0: concepts + initial x ----------------
            for t in range(TILES):
                # relations: rows [rel_emb(32) | srel2(8) | pad]
                rit = load_idx(ridx, t, 128)
                grel = gsm.tile([128, 17 * 64], F32, tag="gsm")
                nc.gpsimd.dma_gather(
                    out_ap=grel[:, :NI // 128 * 64].rearrange("p (m d) -> p m d", d=64),
                    in_ap=rel_tab[:], idxs_ap=rit[:, :NI // 16],
                    num_idxs=NI, num_idxs_reg=NI, elem_size=64,
                    single_packet=False,
                )
                # crel = relu(sum_j rel[:, f] / 16): view (f, j) reduce X
                tcr = smp.tile([128, 32], F32, tag="c32")
                nc.vector.reduce_sum(
                    out=tcr[:],
                    in_=_ap(grel, 1088, 0, [[1, 32], [64, 16]]),
                    axis=mybir.AxisListType.X,
                )
                nc.scalar.activation(
                    out=crel_all[:, t * 32:(t + 1) * 32], in_=tcr[:],
                    func=AF.Relu, scale=1.0 / DEG,
                )
                # per-edge srel2 for both layers -> srel_all[(t,j,lh)]
                nc.vector.tensor_copy(
                    out=_ap(srel_all, W_E, t * 128, [[8, 16], [1, 8]]),
                    in_=_ap(grel, 1088, 32, [[64, 16], [1, 8]]),
                )

                # attributes
                ait = load_idx(aidx, t, 32)
                gat = gsm.tile([128, 256], F32, tag="gsma")
                nc.gpsimd.dma_gather(
                    out_ap=gat[:].rearrange("p (m d) -> p m d", d=64),
                    in_ap=attr_tab[:], idxs_ap=ait[:, :32],
                    num_idxs=NIA, num_idxs_reg=NIA, elem_size=64,
                    single_packet=False,
                )
                tca = smp.tile([128, 32], F32, tag="c32b")
                nc.vector.reduce_sum(
                    out=tca[:],
                    in_=_ap(gat, 256, 0, [[1, 32], [64, 4]]),
                    axis=mybir.AxisListType.X,
                )
                nc.scalar.activation(
                    out=cattr_all[:, t * 32:(t + 1) * 32], in_=tca[:],
                    func=AF.Relu, scale=1.0 / ADEG,
                )

                # initial x: mean of ent[col], then e0 = relu(x0)
                cit = load_idx(cidx, t, NI2 // 16)
                g = gbig.tile([128, 17 * DOUT], F32, tag="g")
                nc.gpsimd.dma_gather(
                    out_ap=g[:].rearrange("p (m d) -> p m d", d=DOUT),
                    in_ap=ent_region[HALF:, :], idxs_ap=cit[:, :NI2 // 16],
                    num_idxs=NI2, num_idxs_reg=NI2, elem_size=DOUT,
                    single_packet=False,
                )
                wg = wgp.tile([128, 2048], F32, tag="wg")
                nc.vector.tensor_add(wg[:, 0:1024], g[:, 0:1024], g[:, 1024:2048])
                nc.vector.tensor_add(wg[:, 0:512], wg[:, 0:512], wg[:, 512:1024])
                nc.vector.tensor_add(wg[:, 0:256], wg[:, 0:256], wg[:, 256:512])
                nc.vector.tensor_add(wg[:, 0:128], wg[:, 0:128], wg[:, 128:256])
                nc.scalar.activation(
                    out=e_all[:, t * 128:(t + 1) * 128], in_=wg[:, 0:128],
                    func=AF.Relu, scale=1.0 / DEG,
                )

            # ---------------- layers ----------------
            for l in range(L):
                kb = l * 6 * 128
                # broadcast-load this layer's kernel vectors [1,768] -> [128,768]
                nc.sync.dma_start(
                    kv_b[:],
                    bass.AP(tensor=kvecs[:].tensor, offset=kb,
                            ap=[[0, 128], [1, 768]]),
                )

                # --- per-node scores SELF/NEIGH via whole-buffer dots ---
                ev = _ap(e_all, W_E, 0, [[128, TILES], [32, 4], [1, 32]])
                dv = _ap(d_all, W_E, 0, [[128, TILES], [32, 4], [1, 32]])
                crv = _ap(crel_all, W_C, 0, [[32, TILES], [0, 4], [1, 32]])
                cav = _ap(cattr_all, W_C, 0, [[32, TILES], [0, 4], [1, 32]])

                def kvv(row):
                    return _ap(kv_b, 768, row * 128, [[0, TILES], [32, 4], [1, 32]])

                def dot_accum(dst, srcs):
                    # srcs: list of (in0_view, kv_row)
                    first = True
                    for in0, krow in srcs:
                        nc.vector.tensor_tensor(out=dv, in0=in0, in1=kvv(krow), op=OP.mult)
                        tgt = dst[:] if first else t196[:]
                        nc.vector.reduce_sum(
                            out=tgt,
                            in_=dv,
                            axis=mybir.AxisListType.X,
                        )
                        if not first:
                            nc.vector.tensor_add(dst[:], dst[:], t196[:])
                        first = False

                dot_accum(self_all, [(ev, 0), (crv, 1), (cav, 2)])
                dot_accum(neigh_all, [(crv, 3), (cav, 4), (ev, 5)])

                # --- write NEIGH and e to DRAM for the all-gathers ---
                nc.sync.dma_start(
                    bass.AP(tensor=agn_in[:].tensor, offset=0,
                            ap=[[4, 128], [TP * 4, TILES - 1], [1, 4]]),
                    _ap(neigh_all, W_4, 0, [[4, TILES - 1], [1, 4]]),
                )
                nc.sync.dma_start(
                    bass.AP(tensor=agn_in[:].tensor, offset=(TILES - 1) * TP * 4,
                            ap=[[4, LASTP], [1, 4]]),
                    _ap(neigh_all, W_4, (TILES - 1) * 4, [[1, 4]], nparts=LASTP),
                )
                nc.sync.dma_start(
                    bass.AP(tensor=agx_in[:].tensor, offset=0,
                            ap=[[128, 128], [TP * 128, TILES - 1], [1, 128]]),
                    _ap(e_all, W_E, 0, [[128, TILES - 1], [1, 128]]),
                )
                nc.sync.dma_start(
                    bass.AP(tensor=agx_in[:].tensor, offset=(TILES - 1) * TP * 128,
                            ap=[[128, LASTP], [1, 128]]),
                    _ap(e_all, W_E, (TILES - 1) * 128, [[1, 128]], nparts=LASTP),
                )

                nc.gpsimd.collective_compute(
                    "AllGather", OP.bypass, replica_groups=RG,
                    ins=[agn_in[:]], outs=[n_full[l][:]],
                )
                nc.gpsimd.collective_compute(
                    "AllGather", OP.bypass, replica_groups=RG,
                    ins=[agx_in[:]], outs=[region_x[l][HALF:, :]],
                )
                # expand neigh into padded region + duplicate blocks
                nc.sync.dma_start(region_n[l][HALF:, 0:4], n_full[l][:, :])
                dupn = nc.sync.dma_start(region_n[l][0:EXTRA, 0:4], n_full[l][HALF:N, :])
                dupx = nc.sync.dma_start(region_x[l][0:EXTRA, :], region_x[l][2 * HALF:RX, :])

                # --- pass 1: per-edge NEIGH gather -> scores ---
                for t in range(TILES):
                    cit = load_idx(cidx, t, NI2 // 16)
                    gn = gsm.tile([128, 17 * 64], F32, tag="gsm")
                    gni = nc.gpsimd.dma_gather(
                        out_ap=gn[:].rearrange("p (m d) -> p m d", d=64),
                        in_ap=region_n[l][HALF:, :], idxs_ap=cit[:, :NI2 // 16],
                        num_idxs=NI2, num_idxs_reg=NI2, elem_size=64,
                        single_packet=False,
                    )
                    # declared in_ap doesn't cover the wraparound duplicate rows
                    add_dep_helper(gni.ins, dupn.ins, sync=True,
                                   reason="neigh wraparound dup before gather")
                    nc.vector.tensor_copy(
                        out=_ap(z_all, W_S, t * 64, [[4, 16], [1, 4]]),
                        in_=_ap(gn, 1088, 0, [[64, 16], [1, 4]]),
                    )

                # s = n + srel[l] + self, then leaky_relu
                s_tjh = _ap(s_all, W_S, 0, [[64, TILES], [4, 16], [1, 4]])
                nc.vector.tensor_tensor(
                    out=s_tjh,
                    in0=_ap(z_all, W_S, 0, [[64, TILES], [4, 16], [1, 4]]),
                    in1=_ap(srel_all, W_E, l * 4, [[128, TILES], [8, 16], [1, 4]]),
                    op=OP.add,
                )
                nc.vector.tensor_tensor(
                    out=s_tjh, in0=s_tjh,
                    in1=_ap(self_all, W_4, 0, [[4, TILES], [0, 16], [1, 4]]),
                    op=OP.add,
                )
                nc.scalar.activation(out=z_all[:], in_=s_all[:], func=AF.Relu, scale=0.7)
                nc.vector.scalar_tensor_tensor(
                    out=s_all[:], in0=s_all[:], scalar=0.3, in1=z_all[:],
                    op0=OP.mult, op1=OP.add,
                )
                # mask pad rows of the last tile (additive -1e9 on pad partitions)
                lastc = (TILES - 1) * 64
                nc.vector.tensor_tensor(
                    out=s_all[:, lastc:lastc + 64],
                    in0=s_all[:, lastc:lastc + 64],
                    in1=pm[:, 0:1].to_broadcast([128, 64]),
                    op=OP.add,
                )

                # --- global softmax stats ---
                nc.vector.reduce_max(
                    out=m4a[:],
                    in_=_ap(s_all, W_S, 0, [[1, 4], [64, TILES], [4, 16]]),
                    axis=mybir.AxisListType.XY,
                )
                nc.gpsimd.partition_all_reduce(
                    m4a2[:], m4a[:], channels=128, reduce_op=bass_isa.ReduceOp.max)
                # z = exp(s - mloc)
                nc.vector.tensor_tensor(
                    out=_ap(z_all, W_S, 0, [[64, TILES], [4, 16], [1, 4]]),
                    in0=s_tjh,
                    in1=_ap(m4a2, 4, 0, [[0, TILES], [0, 16], [1, 4]]),
                    op=OP.subtract,
                )
                nc.scalar.activation(out=z_all[:], in_=z_all[:], func=AF.Exp)
                nc.vector.reduce_sum(
                    out=m4b[:],
                    in_=_ap(z_all, W_S, 0, [[1, 4], [64, TILES], [4, 16]]),
                    axis=mybir.AxisListType.XY,
                )
                nc.gpsimd.partition_all_reduce(
                    m4b2[:], m4b[:], channels=128, reduce_op=bass_isa.ReduceOp.add)

                # all-gather (mloc, ssum) and combine: B = M + ln(Z)
                nc.vector.tensor_copy(stats_sb[0:1, 0:4], m4a2[0:1, :])
                nc.vector.tensor_copy(stats_sb[0:1, 4:8], m4b2[0:1, :])
                nc.sync.dma_start(stats_in[:], stats_sb[:])
                nc.gpsimd.collective_compute(
                    "AllGather", OP.bypass, replica_groups=RG,
                    ins=[stats_in[:]], outs=[stats_full[l][:]],
                )
                nc.sync.dma_start(st8[:], stats_full[l][:])
                nc.gpsimd.partition_all_reduce(
                    sb8a[:], st8[:, 0:4], channels=8, reduce_op=bass_isa.ReduceOp.max)
                nc.vector.tensor_sub(sb8b[:], st8[:, 0:4], sb8a[:])
                nc.scalar.activation(out=sb8b[:], in_=sb8b[:], func=AF.Exp)
                nc.vector.tensor_mul(sb8b[:], sb8b[:], st8[:, 4:8])
                nc.gpsimd.partition_all_reduce(
                    sb8c[:], sb8b[:], channels=8, reduce_op=bass_isa.ReduceOp.add)
                nc.scalar.activation(out=sb8c[:], in_=sb8c[:], func=AF.Ln)
                nc.vector.tensor_add(sb8c[:], sb8c[:], sb8a[:])   # B per-partition(8)
                nc.gpsimd.partition_broadcast(m4c[:], sb8c[0:1, :])
                nc.vector.tensor_sub(m4e[:], m4a2[:], m4c[:])
                nc.scalar.activation(out=m4d[:], in_=m4e[:], func=AF.Exp)

                # s' = z * c ; per-row softmax -> a
                nc.vector.tensor_tensor(
                    out=s_tjh,
                    in0=_ap(z_all, W_S, 0, [[64, TILES], [4, 16], [1, 4]]),
                    in1=_ap(m4d, 4, 0, [[0, TILES], [0, 16], [1, 4]]),
                    op=OP.mult,
                )
                nc.vector.reduce_max(
                    out=mrow[:],
                    in_=_ap(s_all, W_S, 0, [[64, TILES], [1, 4], [4, 16]]),
                    axis=mybir.AxisListType.X,
                )
                nc.vector.tensor_tensor(
                    out=_ap(z_all, W_S, 0, [[64, TILES], [4, 16], [1, 4]]),
                    in0=s_tjh,
                    in1=_ap(mrow, W_4, 0, [[4, TILES], [0, 16], [1, 4]]),
                    op=OP.subtract,
                )
                nc.scalar.activation(out=z_all[:], in_=z_all[:], func=AF.Exp)
                nc.vector.reduce_sum(
                    out=den[:],
                    in_=_ap(z_all, W_S, 0, [[64, TILES], [1, 4], [4, 16]]),
                    axis=mybir.AxisListType.X,
                )
                nc.vector.reciprocal(out=den[:], in_=den[:])
                nc.vector.tensor_tensor(
                    out=_ap(a_all, W_S, 0, [[64, TILES], [4, 16], [1, 4]]),
                    in0=_ap(z_all, W_S, 0, [[64, TILES], [4, 16], [1, 4]]),
                    in1=_ap(den, W_4, 0, [[4, TILES], [0, 16], [1, 4]]),
                    op=OP.mult,
                )

                if debug and l == 0:
                    dt_ = smp.tile([128, 448], F32, tag="dbgt")
                    nc.vector.tensor_copy(dt_[:, 0:4], m4a2[:])
                    nc.vector.tensor_copy(dt_[:, 4:8], m4b2[:])
                    nc.vector.tensor_copy(dt_[:, 8:12], m4c[:])
                    nc.vector.tensor_copy(dt_[:, 12:16], m4d[:])
                    nc.vector.tensor_copy(dt_[:, 16:80], s_all[:, 0:64])      # s' tile0
                    nc.vector.tensor_copy(dt_[:, 80:144], a_all[:, 0:64])     # a tile0
                    nc.vector.tensor_copy(dt_[:, 144:176], crel_all[:, 0:32])
                    nc.vector.tensor_copy(dt_[:, 176:208], cattr_all[:, 0:32])
                    nc.vector.tensor_copy(dt_[:, 208:336], e_all[:, 0:128])   # e0 tile0
                    nc.vector.tensor_copy(dt_[:, 336:340], self_all[:, 0:4])
                    nc.vector.tensor_copy(dt_[:, 340:344], neigh_all[:, 0:4])
                    nc.vector.tensor_copy(dt_[:, 344:408], z_all[:, 0:64])
                    nc.vector.tensor_copy(dt_[:, 408:440], srel_all[:, 0:32])
                    nc.sync.dma_start(dbg[:, 0:448], dt_[:])

                # --- pass 2: gather e[col], weight, aggregate, tanh ---
                for t in range(TILES):
                    cit = load_idx(cidx, t, NI2 // 16)
                    g = gbig.tile([128, 17 * DOUT], F32, tag="g")
                    gxi = nc.gpsimd.dma_gather(
                        out_ap=g[:].rearrange("p (m d) -> p m d", d=DOUT),
                        in_ap=region_x[l][HALF:, :], idxs_ap=cit[:, :NI2 // 16],
                        num_idxs=NI2, num_idxs_reg=NI2, elem_size=DOUT,
                        single_packet=False,
                    )
                    add_dep_helper(gxi.ins, dupx.ins, sync=True,
                                   reason="x wraparound dup before gather")
                    wg = wgp.tile([128, 2048], F32, tag="wg")
                    for h in range(H):
                        nc.vector.tensor_tensor(
                            out=_ap(wg, 2048, h * 32, [[128, 16], [1, 32]]),
                            in0=_ap(g, 2176, h * 32, [[128, 16], [1, 32]]),
                            in1=_ap(a_all, W_S, t * 64 + h, [[4, 16], [0, 32]]),
                            op=OP.mult,
                        )
                    nc.vector.tensor_add(wg[:, 0:1024], wg[:, 0:1024], wg[:, 1024:2048])
                    nc.vector.tensor_add(wg[:, 0:512], wg[:, 0:512], wg[:, 512:1024])
                    nc.vector.tensor_add(wg[:, 0:256], wg[:, 0:256], wg[:, 256:512])
                    nc.vector.tensor_add(wg[:, 0:128], wg[:, 0:128], wg[:, 128:256])
                    xt = smp.tile([128, 128], F32, tag="xt")
                    nc.scalar.activation(out=xt[:], in_=wg[:, 0:128], func=AF.Tanh)
                    if debug and l == 0 and t == 0:
                        dt2 = smp.tile([128, 256], F32, tag="dbgt2")
                        nc.vector.tensor_copy(dt2[:, 0:128], g[:, 0:128])   # e_full[cols[p,0]]
                        nc.vector.tensor_copy(dt2[:, 128:256], xt[:])
                        nc.sync.dma_start(dbg[:, 448:704], dt2[:])
                    rp = TP if t < TILES - 1 else LASTP
                    nc.sync.dma_start(
                        bass.AP(tensor=out[:].tensor,
                                offset=t * TP * (L * DOUT) + l * DOUT,
                                ap=[[L * DOUT, rp], [1, DOUT]]),
                        xt[:rp, :],
                    )
                    if l + 1 < L:
                        nc.scalar.activation(
                            out=e_all[:, t * 128:(t + 1) * 128], in_=xt[:],
                            func=AF.Relu,
                        )

    nc.finalize()
    return nc


def _host_prepare(inputs):
    ent = np.asarray(inputs["ent_emb"], dtype=np.float32)
    rel = np.asarray(inputs["rel_emb"], dtype=np.float32)
    attr = np.asarray(inputs["attr_emb"], dtype=np.float32)
    kern = np.asarray(inputs["attn_kernels"], dtype=np.float32)
    edge_index = np.asarray(inputs["edge_index"], dtype=np.int64)
    edge_rel = np.asarray(inputs["edge_rel"], dtype=np.int64)
    attr_index = np.asarray(inputs["attr_index"], dtype=np.int64)

    # region with wraparound duplicate for int16 sign trick
    ent_region = np.zeros((RX, DOUT), dtype=np.float32)
    ent_region[HALF:] = ent
    ent_region[:EXTRA] = ent[HALF:]

    # rel table rows: [rel_emb(32) | srel2 per (l,h) (8) | pad(24)]
    rel_tab = np.zeros((R, 64), dtype=np.float32)
    rel_tab[:, 0:32] = rel
    for l in range(L):
        for h in range(H):
            rel_tab[:, 32 + l * 4 + h] = rel @ kern[l, h, 96:128]

    attr_tab = np.zeros((A, 64), dtype=np.float32)
    attr_tab[:, 0:32] = attr

    kv = np.zeros((L, 6, 128), dtype=np.float32)
    for l in range(L):
        kv[l, 0] = kern[l, :, 0:32].reshape(-1)
        kv[l, 1] = kern[l, :, 32:64].reshape(-1)
        kv[l, 2] = kern[l, :, 64:96].reshape(-1)
        kv[l, 3] = kern[l, :, 128:160].reshape(-1)
        kv[l, 4] = kern[l, :, 160:192].reshape(-1)
        kv[l, 5] = kern[l, :, 192:224].reshape(-1)
    kvecs = kv.reshape(1, -1)

    def pack(vals, deg, tail_pad=False):
        # vals: [NB, deg] int64 per core -> [TILES, 128, cols] int16
        padded = np.zeros((NPAD, deg), dtype=np.int64)
        padded[:NB] = vals
        v = padded.reshape(TILES, TP, deg).transpose(0, 2, 1).reshape(TILES, TP * deg)
        p16 = v.reshape(TILES, (TP * deg) // 16, 16).transpose(0, 2, 1)
        p16 = np.where(p16 < HALF, p16, p16 - 65536).astype(np.int16)
        if tail_pad:
            # one extra all-zero index column: the gather stream must not end
            # on a negative (sign-wrapped) index or the firmware drops the tail
            p16 = np.concatenate(
                [p16, np.zeros((TILES, 16, 1), np.int16)], axis=2)
        return np.tile(p16, (1, 8, 1))

    padmask = np.zeros((128, 1), dtype=np.float32)
    padmask[LASTP:, 0] = NEG_BIG

    cols = edge_index[:, 1].reshape(N, DEG)
    rels = edge_rel.reshape(N, DEG)
    aids = attr_index[:, 1].reshape(N, ADEG)

    in_maps = []
    for c in range(NCORES):
        lo, hi = c * NB, (c + 1) * NB
        in_maps.append({
            "ent_region": ent_region,
            "rel_tab": rel_tab,
            "attr_tab": attr_tab,
            "kvecs": kvecs,
            "padmask": padmask,
            "cidx": pack(cols[lo:hi], DEG, tail_pad=True),
            "ridx": pack(rels[lo:hi], DEG),
            "aidx": pack(aids[lo:hi], ADEG),
        })
    return in_maps


def kernel(**inputs):
    if "nc" not in _cached:
        _cached["nc"] = _build()
    nc = _cached["nc"]
    in_maps = _host_prepare(inputs)
    res = run_bass_kernel_spmd(nc, in_maps, core_ids=list(range(NCORES)))
    out = np.concatenate([res.results[c]["out"] for c in range(NCORES)], axis=0)
    return out


def kernel_debug(**inputs):
    if "ncd" not in _cached:
        _cached["ncd"] = _build(debug=True)
    nc = _cached["ncd"]
    in_maps = _host_prepare(inputs)
    res = run_bass_kernel_spmd(nc, in_maps, core_ids=list(range(NCORES)))
    out = np.concatenate([res.results[c]["out"] for c in range(NCORES)], axis=0)
    return out, [res.results[c]["dbg"] for c in range(NCORES)]


# revision 24
# speedup vs baseline: 70.7462x; 70.7462x over previous
"""GCAT (graph attention over ent/rel/attr embeddings) on 8 Trainium2 cores.

Sharding: edges are grouped 16-per-node and node-sorted, so we shard nodes
(and thus edges) into 8 contiguous blocks, one per core.  Embedding tables
and attention kernels are replicated.  Per layer each core computes its
nodes' features, all-gathers the [N,128] feature table (+ the [N,4]
per-node neighbor score table), then does per-edge gathers from the
replicated tables with dma_gather.

dma_gather indices are int16 (signed).  Rows >= 32768 are addressed via a
sign-wraparound trick: the gather base points at row 32768 of a region
whose first 17232 rows duplicate table rows 32768..49999, so negative
int16 indices (col - 65536) land on the duplicate copy.
"""

import sys

sys.path.insert(0, "/opt/trn_rl_repo")

import numpy as np

import concourse.bacc as bacc
import concourse.bass as bass
import concourse.bass_isa as bass_isa
import concourse.tile as tile
from concourse import mybir
from concourse.bass_utils import run_bass_kernel_spmd
from concourse.tile_rust import add_dep_helper

F32 = mybir.dt.float32
I16 = mybir.dt.int16
AF = mybir.ActivationFunctionType
OP = mybir.AluOpType

NCORES = 8
N = 50000
DEG = 16
E = N * DEG
R = 1000
A = 2000
ADEG = 4
DOUT = 128
H = 4
DH = 32
L = 2
NB = N // NCORES          # 6250 nodes per core
EB = NB * DEG             # 100000 edges per core
TP = 128                  # nodes per tile
TILES = (NB + TP - 1) // TP   # 49
NPAD = TILES * TP         # 6272
LASTP = NB - (TILES - 1) * TP  # 106 valid partitions in the last tile
HALF = 32768
EXTRA = N - HALF          # 17232 duplicated rows
RX = HALF + N             # 82768 region rows
NI = TP * DEG             # 2048 gather indices per tile
NI2 = NI + 16             # +16 zero-index pad so the stream never ends negative
                          # (the Q7 firmware drops a trailing run of negative
                          # int16 indices; mid-stream negatives are processed)
NIA = TP * ADEG           # 512 attr indices per tile
NEG_BIG = -1.0e9

_cached = {}


def _ap(t, width, offset, pairs, nparts=128, poff=0):
    """Strided view of a [nparts, width] SBUF tile. pairs = [[step, count], ...]"""
    base = t[:]
    return bass.AP(
        tensor=base.tensor,
        offset=base.offset + poff * width + offset,
        ap=[[width, nparts]] + [list(p) for p in pairs],
    )


def _build(debug=False):
    nc = bacc.Bacc(num_devices=NCORES)
    dbg = None
    if debug:
        dbg = nc.dram_tensor("dbg", [128, 704], F32, kind="ExternalOutput")

    ent_shard = nc.dram_tensor("ent_shard", [NB, DOUT], F32, kind="ExternalInput")
    rel_tab = nc.dram_tensor("rel_tab", [R, 64], F32, kind="ExternalInput")
    attr_tab = nc.dram_tensor("attr_tab", [A, 64], F32, kind="ExternalInput")
    kvecs = nc.dram_tensor("kvecs", [1, L * 6 * 128], F32, kind="ExternalInput")
    cidx = nc.dram_tensor("cidx", [TILES, 128, NI2 // 16], I16, kind="ExternalInput")
    ridx = nc.dram_tensor("ridx", [TILES, 128, NI // 16], I16, kind="ExternalInput")
    aidx = nc.dram_tensor("aidx", [TILES, 128, NIA // 16], I16, kind="ExternalInput")
    padmask = nc.dram_tensor("padmask", [128, 1], F32, kind="ExternalInput")
    out = nc.dram_tensor("out", [NB, L * DOUT], F32, kind="ExternalOutput")

    # Internal DRAM (per layer to avoid cross-layer races through collectives)
    region_x = [nc.dram_tensor(f"region_x{i}", [RX, DOUT], F32, kind="Internal",
                               addr_space="Shared") for i in range(L)]
    region_n = [nc.dram_tensor(f"region_n{i}", [RX, 64], F32, kind="Internal")
                for i in range(L)]
    n_full = [nc.dram_tensor(f"n_full{i}", [N, 4], F32, kind="Internal",
                             addr_space="Shared") for i in range(L)]
    stats_full = [nc.dram_tensor(f"stats_full{i}", [NCORES, 8], F32, kind="Internal",
                                 addr_space="Shared") for i in range(L)]
    region_e = nc.dram_tensor("region_e", [RX, DOUT], F32, kind="Internal",
                              addr_space="Shared")
    ent_bounce = nc.dram_tensor("ent_bounce", [NB, DOUT], F32, kind="Internal")
    agx_in = nc.dram_tensor("agx_in", [NB, DOUT], F32, kind="Internal")
    agn_in = nc.dram_tensor("agn_in", [NB, 4], F32, kind="Internal")
    stats_in = nc.dram_tensor("stats_in", [1, 8], F32, kind="Internal")

    RG = [list(range(NCORES))]

    with tile.TileContext(nc) as tc:
        with (
            tc.tile_pool(name="per", bufs=1) as per,          # persistent buffers
            tc.tile_pool(name="gbig", bufs=2) as gbig,        # 2048-col gather tiles
            tc.tile_pool(name="gsm", bufs=2) as gsm,          # 1024-col gather tiles
            tc.tile_pool(name="wgp", bufs=2) as wgp,          # weighted/tree scratch
            tc.tile_pool(name="idxp", bufs=2) as idxp,        # idx tiles
            tc.tile_pool(name="smp", bufs=2) as smp,          # small scratch
        ):
            W_E = TILES * 128      # 6272
            W_S = TILES * 64       # 3136
            W_C = TILES * 32       # 1568
            W_4 = TILES * 4        # 196

            e_all = per.tile([128, W_E], F32, tag="e_all")
            d_all = per.tile([128, W_E], F32, tag="d_all")
            srel_all = per.tile([128, W_E], F32, tag="srel_all")
            crel_all = per.tile([128, W_C], F32, tag="crel_all")
            cattr_all = per.tile([128, W_C], F32, tag="cattr_all")
            s_all = per.tile([128, W_S], F32, tag="s_all")
            z_all = per.tile([128, W_S], F32, tag="z_all")   # also: n_all, leaky scratch
            a_all = per.tile([128, W_S], F32, tag="a_all")
            self_all = per.tile([128, W_4], F32, tag="self_all")
            neigh_all = per.tile([128, W_4], F32, tag="neigh_all")
            t196 = per.tile([128, W_4], F32, tag="t196")
            mrow = per.tile([128, W_4], F32, tag="mrow")
            den = per.tile([128, W_4], F32, tag="den")
            kv_b = per.tile([128, 768], F32, tag="kv_b")
            m4a = per.tile([128, 4], F32, tag="m4a")   # mloc
            m4b = per.tile([128, 4], F32, tag="m4b")   # ssum
            m4c = per.tile([128, 4], F32, tag="m4c")   # B
            m4d = per.tile([128, 4], F32, tag="m4d")   # c = exp(mloc - B)
            m4e = per.tile([128, 4], F32, tag="m4e")   # scratch
            m4a2 = per.tile([128, 4], F32, tag="m4a2")
            m4b2 = per.tile([128, 4], F32, tag="m4b2")
            sb8c = per.tile([8, 4], F32, tag="sb8c")
            st8 = per.tile([8, 8], F32, tag="st8")
            sb8a = per.tile([8, 4], F32, tag="sb8a")
            sb8b = per.tile([8, 4], F32, tag="sb8b")
            stats_sb = per.tile([1, 8], F32, tag="stats_sb")
            pm = per.tile([128, 1], F32, tag="pm")
            nc.sync.dma_start(pm[:], padmask[:])

            def load_idx(src, t, cols):
                it = idxp.tile([128, 132], I16, tag="idx")
                nc.sync.dma_start(it[:, :cols], src[t, :, :])
                return it

            # build the ent gather region on device from the sharded input
            nc.sync.dma_start(ent_bounce[:], ent_shard[:])
            nc.gpsimd.collective_compute(
                "AllGather", OP.bypass, replica_groups=RG,
                ins=[ent_bounce[:]], outs=[region_e[HALF:, :]],
            )
            dupe = nc.sync.dma_start(region_e[0:EXTRA, :], region_e[2 * HALF:RX, :])

            # ---------------- phase 0: concepts + initial x ----------------
            for t in range(TILES):
                # relations: rows [rel_emb(32) | srel2(8) | pad]
                rit = load_idx(ridx, t, 128)
                grel = gsm.tile([128, 17 * 64], F32, tag="gsm")
                nc.gpsimd.dma_gather(
                    out_ap=grel[:, :NI // 128 * 64].rearrange("p (m d) -> p m d", d=64),
                    in_ap=rel_tab[:], idxs_ap=rit[:, :NI // 16],
                    num_idxs=NI, num_idxs_reg=NI, elem_size=64,
                    single_packet=False,
                )
                # crel = relu(sum_j rel[:, f] / 16): view (f, j) reduce X
                tcr = smp.tile([128, 32], F32, tag="c32")
                nc.vector.reduce_sum(
                    out=tcr[:],
                    in_=_ap(grel, 1088, 0, [[1, 32], [64, 16]]),
                    axis=mybir.AxisListType.X,
                )
                nc.scalar.activation(
                    out=crel_all[:, t * 32:(t + 1) * 32], in_=tcr[:],
                    func=AF.Relu, scale=1.0 / DEG,
                )
                # per-edge srel2 for both layers -> srel_all[(t,j,lh)]
                nc.vector.tensor_copy(
                    out=_ap(srel_all, W_E, t * 128, [[8, 16], [1, 8]]),
                    in_=_ap(grel, 1088, 32, [[64, 16], [1, 8]]),
                )

                # attributes
                ait = load_idx(aidx, t, 32)
                gat = gsm.tile([128, 256], F32, tag="gsma")
                nc.gpsimd.dma_gather(
                    out_ap=gat[:].rearrange("p (m d) -> p m d", d=64),
                    in_ap=attr_tab[:], idxs_ap=ait[:, :32],
                    num_idxs=NIA, num_idxs_reg=NIA, elem_size=64,
                    single_packet=False,
                )
                tca = smp.tile([128, 32], F32, tag="c32b")
                nc.vector.reduce_sum(
                    out=tca[:],
                    in_=_ap(gat, 256, 0, [[1, 32], [64, 4]]),
                    axis=mybir.AxisListType.X,
                )
                nc.scalar.activation(
                    out=cattr_all[:, t * 32:(t + 1) * 32], in_=tca[:],
                    func=AF.Relu, scale=1.0 / ADEG,
                )

                # initial x: mean of ent[col], then e0 = relu(x0)
                cit = load_idx(cidx, t, NI2 // 16)
                g = gbig.tile([128, 17 * DOUT], F32, tag="g")
                gei = nc.gpsimd.dma_gather(
                    out_ap=g[:].rearrange("p (m d) -> p m d", d=DOUT),
                    in_ap=region_e[HALF:, :], idxs_ap=cit[:, :NI2 // 16],
                    num_idxs=NI2, num_idxs_reg=NI2, elem_size=DOUT,
                    single_packet=False,
                )
                add_dep_helper(gei.ins, dupe.ins, sync=True,
                               reason="ent wraparound dup before gather")
                wg = wgp.tile([128, 2048], F32, tag="wg")
                nc.vector.tensor_add(wg[:, 0:1024], g[:, 0:1024], g[:, 1024:2048])
                nc.vector.tensor_add(wg[:, 0:512], wg[:, 0:512], wg[:, 512:1024])
                nc.vector.tensor_add(wg[:, 0:256], wg[:, 0:256], wg[:, 256:512])
                nc.vector.tensor_add(wg[:, 0:128], wg[:, 0:128], wg[:, 128:256])
                nc.scalar.activation(
                    out=e_all[:, t * 128:(t + 1) * 128], in_=wg[:, 0:128],
                    func=AF.Relu, scale=1.0 / DEG,
                )

            # ---------------- layers ----------------
            for l in range(L):
                kb = l * 6 * 128
                # broadcast-load this layer's kernel vectors [1,768] -> [128,768]
                nc.sync.dma_start(
                    kv_b[:],
                    bass.AP(tensor=kvecs[:].tensor, offset=kb,
                            ap=[[0, 128], [1, 768]]),
                )

                # --- per-node scores SELF/NEIGH via whole-buffer dots ---
                ev = _ap(e_all, W_E, 0, [[128, TILES], [32, 4], [1, 32]])
                dv = _ap(d_all, W_E, 0, [[128, TILES], [32, 4], [1, 32]])
                crv = _ap(crel_all, W_C, 0, [[32, TILES], [0, 4], [1, 32]])
                cav = _ap(cattr_all, W_C, 0, [[32, TILES], [0, 4], [1, 32]])

                def kvv(row):
                    return _ap(kv_b, 768, row * 128, [[0, TILES], [32, 4], [1, 32]])

                def dot_accum(dst, srcs):
                    # srcs: list of (in0_view, kv_row)
                    first = True
                    for in0, krow in srcs:
                        nc.vector.tensor_tensor(out=dv, in0=in0, in1=kvv(krow), op=OP.mult)
                        tgt = dst[:] if first else t196[:]
                        nc.vector.reduce_sum(
                            out=tgt,
                            in_=dv,
                            axis=mybir.AxisListType.X,
                        )
                        if not first:
                            nc.vector.tensor_add(dst[:], dst[:], t196[:])
                        first = False

                dot_accum(self_all, [(ev, 0), (crv, 1), (cav, 2)])
                dot_accum(neigh_all, [(crv, 3), (cav, 4), (ev, 5)])

                # --- write NEIGH and e to DRAM for the all-gathers ---
                nc.sync.dma_start(
                    bass.AP(tensor=agn_in[:].tensor, offset=0,
                            ap=[[4, 128], [TP * 4, TILES - 1], [1, 4]]),
                    _ap(neigh_all, W_4, 0, [[4, TILES - 1], [1, 4]]),
                )
                nc.sync.dma_start(
                    bass.AP(tensor=agn_in[:].tensor, offset=(TILES - 1) * TP * 4,
                            ap=[[4, LASTP], [1, 4]]),
                    _ap(neigh_all, W_4, (TILES - 1) * 4, [[1, 4]], nparts=LASTP),
                )
                nc.sync.dma_start(
                    bass.AP(tensor=agx_in[:].tensor, offset=0,
                            ap=[[128, 128], [TP * 128, TILES - 1], [1, 128]]),
                    _ap(e_all, W_E, 0, [[128, TILES - 1], [1, 128]]),
                )
                nc.sync.dma_start(
                    bass.AP(tensor=agx_in[:].tensor, offset=(TILES - 1) * TP * 128,
                            ap=[[128, LASTP], [1, 128]]),
                    _ap(e_all, W_E, (TILES - 1) * 128, [[1, 128]], nparts=LASTP),
                )

                nc.gpsimd.collective_compute(
                    "AllGather", OP.bypass, replica_groups=RG,
                    ins=[agn_in[:]], outs=[n_full[l][:]],
                )
                nc.gpsimd.collective_compute(
                    "AllGather", OP.bypass, replica_groups=RG,
                    ins=[agx_in[:]], outs=[region_x[l][HALF:, :]],
                )
                # expand neigh into padded region + duplicate blocks
                nc.sync.dma_start(region_n[l][HALF:, 0:4], n_full[l][:, :])
                dupn = nc.sync.dma_start(region_n[l][0:EXTRA, 0:4], n_full[l][HALF:N, :])
                dupx = nc.sync.dma_start(region_x[l][0:EXTRA, :], region_x[l][2 * HALF:RX, :])

                # --- pass 1: per-edge NEIGH gather -> scores ---
                for t in range(TILES):
                    cit = load_idx(cidx, t, NI2 // 16)
                    gn = gsm.tile([128, 17 * 64], F32, tag="gsm")
                    gni = nc.gpsimd.dma_gather(
                        out_ap=gn[:].rearrange("p (m d) -> p m d", d=64),
                        in_ap=region_n[l][HALF:, :], idxs_ap=cit[:, :NI2 // 16],
                        num_idxs=NI2, num_idxs_reg=NI2, elem_size=64,
                        single_packet=False,
                    )
                    # declared in_ap doesn't cover the wraparound duplicate rows
                    add_dep_helper(gni.ins, dupn.ins, sync=True,
                                   reason="neigh wraparound dup before gather")
                    nc.vector.tensor_copy(
                        out=_ap(z_all, W_S, t * 64, [[4, 16], [1, 4]]),
                        in_=_ap(gn, 1088, 0, [[64, 16], [1, 4]]),
                    )

                # s = n + srel[l] + self, then leaky_relu
                s_tjh = _ap(s_all, W_S, 0, [[64, TILES], [4, 16], [1, 4]])
                nc.vector.tensor_tensor(
                    out=s_tjh,
                    in0=_ap(z_all, W_S, 0, [[64, TILES], [4, 16], [1, 4]]),
                    in1=_ap(srel_all, W_E, l * 4, [[128, TILES], [8, 16], [1, 4]]),
                    op=OP.add,
                )
                nc.vector.tensor_tensor(
                    out=s_tjh, in0=s_tjh,
                    in1=_ap(self_all, W_4, 0, [[4, TILES], [0, 16], [1, 4]]),
                    op=OP.add,
                )
                nc.scalar.activation(out=z_all[:], in_=s_all[:], func=AF.Relu, scale=0.7)
                nc.vector.scalar_tensor_tensor(
                    out=s_all[:], in0=s_all[:], scalar=0.3, in1=z_all[:],
                    op0=OP.mult, op1=OP.add,
                )
                # mask pad rows of the last tile (additive -1e9 on pad partitions)
                lastc = (TILES - 1) * 64
                nc.vector.tensor_tensor(
                    out=s_all[:, lastc:lastc + 64],
                    in0=s_all[:, lastc:lastc + 64],
                    in1=pm[:, 0:1].to_broadcast([128, 64]),
                    op=OP.add,
                )

                # --- global softmax stats ---
                nc.vector.reduce_max(
                    out=m4a[:],
                    in_=_ap(s_all, W_S, 0, [[1, 4], [64, TILES], [4, 16]]),
                    axis=mybir.AxisListType.XY,
                )
                nc.gpsimd.partition_all_reduce(
                    m4a2[:], m4a[:], channels=128, reduce_op=bass_isa.ReduceOp.max)
                # z = exp(s - mloc)
                nc.vector.tensor_tensor(
                    out=_ap(z_all, W_S, 0, [[64, TILES], [4, 16], [1, 4]]),
                    in0=s_tjh,
                    in1=_ap(m4a2, 4, 0, [[0, TILES], [0, 16], [1, 4]]),
                    op=OP.subtract,
                )
                nc.scalar.activation(out=z_all[:], in_=z_all[:], func=AF.Exp)
                nc.vector.reduce_sum(
                    out=m4b[:],
                    in_=_ap(z_all, W_S, 0, [[1, 4], [64, TILES], [4, 16]]),
                    axis=mybir.AxisListType.XY,
                )
                nc.gpsimd.partition_all_reduce(
                    m4b2[:], m4b[:], channels=128, reduce_op=bass_isa.ReduceOp.add)

                # all-gather (mloc, ssum) and combine: B = M + ln(Z)
                nc.vector.tensor_copy(stats_sb[0:1, 0:4], m4a2[0:1, :])
                nc.vector.tensor_copy(stats_sb[0:1, 4:8], m4b2[0:1, :])
                nc.sync.dma_start(stats_in[:], stats_sb[:])
                nc.gpsimd.collective_compute(
                    "AllGather", OP.bypass, replica_groups=RG,
                    ins=[stats_in[:]], outs=[stats_full[l][:]],
                )
                nc.sync.dma_start(st8[:], stats_full[l][:])
                nc.gpsimd.partition_all_reduce(
                    sb8a[:], st8[:, 0:4], channels=8, reduce_op=bass_isa.ReduceOp.max)
                nc.vector.tensor_sub(sb8b[:], st8[:, 0:4], sb8a[:])
                nc.scalar.activation(out=sb8b[:], in_=sb8b[:], func=AF.Exp)
                nc.vector.tensor_mul(sb8b[:], sb8b[:], st8[:, 4:8])
                nc.gpsimd.partition_all_reduce(
                    sb8c[:], sb8b[:], channels=8, reduce_op=bass_isa.ReduceOp.add)
                nc.scalar.activation(out=sb8c[:], in_=sb8c[:], func=AF.Ln)
                nc.vector.tensor_add(sb8c[:], sb8c[:], sb8a[:])   # B per-partition(8)
                nc.gpsimd.partition_broadcast(m4c[:], sb8c[0:1, :])
                nc.vector.tensor_sub(m4e[:], m4a2[:], m4c[:])
                nc.scalar.activation(out=m4d[:], in_=m4e[:], func=AF.Exp)

                # s' = z * c ; per-row softmax -> a
                nc.vector.tensor_tensor(
                    out=s_tjh,
                    in0=_ap(z_all, W_S, 0, [[64, TILES], [4, 16], [1, 4]]),
                    in1=_ap(m4d, 4, 0, [[0, TILES], [0, 16], [1, 4]]),
                    op=OP.mult,
                )
                nc.vector.reduce_max(
                    out=mrow[:],
                    in_=_ap(s_all, W_S, 0, [[64, TILES], [1, 4], [4, 16]]),
                    axis=mybir.AxisListType.X,
                )
                nc.vector.tensor_tensor(
                    out=_ap(z_all, W_S, 0, [[64, TILES], [4, 16], [1, 4]]),
                    in0=s_tjh,
                    in1=_ap(mrow, W_4, 0, [[4, TILES], [0, 16], [1, 4]]),
                    op=OP.subtract,
                )
                nc.scalar.activation(out=z_all[:], in_=z_all[:], func=AF.Exp)
                nc.vector.reduce_sum(
                    out=den[:],
                    in_=_ap(z_all, W_S, 0, [[64, TILES], [1, 4], [4, 16]]),
                    axis=mybir.AxisListType.X,
                )
                nc.vector.reciprocal(out=den[:], in_=den[:])
                nc.vector.tensor_tensor(
                    out=_ap(a_all, W_S, 0, [[64, TILES], [4, 16], [1, 4]]),
                    in0=_ap(z_all, W_S, 0, [[64, TILES], [4, 16], [1, 4]]),
                    in1=_ap(den, W_4, 0, [[4, TILES], [0, 16], [1, 4]]),
                    op=OP.mult,
                )

                if debug and l == 0:
                    dt_ = smp.tile([128, 448], F32, tag="dbgt")
                    nc.vector.tensor_copy(dt_[:, 0:4], m4a2[:])
                    nc.vector.tensor_copy(dt_[:, 4:8], m4b2[:])
                    nc.vector.tensor_copy(dt_[:, 8:12], m4c[:])
                    nc.vector.tensor_copy(dt_[:, 12:16], m4d[:])
                    nc.vector.tensor_copy(dt_[:, 16:80], s_all[:, 0:64])      # s' tile0
                    nc.vector.tensor_copy(dt_[:, 80:144], a_all[:, 0:64])     # a tile0
                    nc.vector.tensor_copy(dt_[:, 144:176], crel_all[:, 0:32])
                    nc.vector.tensor_copy(dt_[:, 176:208], cattr_all[:, 0:32])
                    nc.vector.tensor_copy(dt_[:, 208:336], e_all[:, 0:128])   # e0 tile0
                    nc.vector.tensor_copy(dt_[:, 336:340], self_all[:, 0:4])
                    nc.vector.tensor_copy(dt_[:, 340:344], neigh_all[:, 0:4])
                    nc.vector.tensor_copy(dt_[:, 344:408], z_all[:, 0:64])
                    nc.vector.tensor_copy(dt_[:, 408:440], srel_all[:, 0:32])
                    nc.sync.dma_start(dbg[:, 0:448], dt_[:])

                # --- pass 2: gather e[col], weight, aggregate, tanh ---
                for t in range(TILES):
                    cit = load_idx(cidx, t, NI2 // 16)
                    g = gbig.tile([128, 17 * DOUT], F32, tag="g")
                    gxi = nc.gpsimd.dma_gather(
                        out_ap=g[:].rearrange("p (m d) -> p m d", d=DOUT),
                        in_ap=region_x[l][HALF:, :], idxs_ap=cit[:, :NI2 // 16],
                        num_idxs=NI2, num_idxs_reg=NI2, elem_size=DOUT,
                        single_packet=False,
                    )
                    add_dep_helper(gxi.ins, dupx.ins, sync=True,
                                   reason="x wraparound dup before gather")
                    wg = wgp.tile([128, 2048], F32, tag="wg")
                    for h in range(H):
                        nc.vector.tensor_tensor(
                            out=_ap(wg, 2048, h * 32, [[128, 16], [1, 32]]),
                            in0=_ap(g, 2176, h * 32, [[128, 16], [1, 32]]),
                            in1=_ap(a_all, W_S, t * 64 + h, [[4, 16], [0, 32]]),
                            op=OP.mult,
                        )
                    nc.vector.tensor_add(wg[:, 0:1024], wg[:, 0:1024], wg[:, 1024:2048])
                    nc.vector.tensor_add(wg[:, 0:512], wg[:, 0:512], wg[:, 512:1024])
                    nc.vector.tensor_add(wg[:, 0:256], wg[:, 0:256], wg[:, 256:512])
                    nc.vector.tensor_add(wg[:, 0:128], wg[:, 0:128], wg[:, 128:256])
                    xt = smp.tile([128, 128], F32, tag="xt")
                    nc.scalar.activation(out=xt[:], in_=wg[:, 0:128], func=AF.Tanh)
                    if debug and l == 0 and t == 0:
                        dt2 = smp.tile([128, 256], F32, tag="dbgt2")
                        nc.vector.tensor_copy(dt2[:, 0:128], g[:, 0:128])   # e_full[cols[p,0]]
                        nc.vector.tensor_copy(dt2[:, 128:256], xt[:])
                        nc.sync.dma_start(dbg[:, 448:704], dt2[:])
                    rp = TP if t < TILES - 1 else LASTP
                    nc.sync.dma_start(
                        bass.AP(tensor=out[:].tensor,
                                offset=t * TP * (L * DOUT) + l * DOUT,
                                ap=[[L * DOUT, rp], [1, DOUT]]),
                        xt[:rp, :],
                    )
                    if l + 1 < L:
                        nc.scalar.activation(
                            out=e_all[:, t * 128:(t + 1) * 128], in_=xt[:],
                            func=AF.Relu,
                        )

    nc.finalize()
    return nc




def _make_runner(nc):
    """Cached shard_map runner for the axon/PJRT path.

    Differences from bass_utils.run_bass_kernel_spmd: the jitted executable
    and the zero output templates are cached across calls, outputs are not
    donated (every element of "out" is written by the kernel), and each
    global output is fetched from device exactly once.
    """
    import jax
    from jax.sharding import Mesh, PartitionSpec
    from jax.experimental.shard_map import shard_map
    from concourse import bass2jax

    bass2jax.install_neuronx_cc_hook()
    partition_name = nc.partition_id_tensor.name if nc.partition_id_tensor else None
    in_names, out_names, out_avals, zero_outs = [], [], [], []
    for alloc in nc.m.functions[0].allocations:
        if not isinstance(alloc, mybir.MemoryLocationSet):
            continue
        name = alloc.memorylocations[0].name
        if alloc.kind == "ExternalInput":
            if name != partition_name:
                in_names.append(name)
        elif alloc.kind == "ExternalOutput":
            out_names.append(name)
            shape = tuple(alloc.tensor_shape)
            dtype = mybir.dt.np(alloc.dtype)
            out_avals.append(jax.core.ShapedArray(shape, dtype))
            zero_outs.append(np.zeros((NCORES * shape[0], *shape[1:]), dtype))
    n_params = len(in_names)
    all_names = list(in_names) + list(out_names)
    if partition_name is not None:
        all_names.append(partition_name)

    def _body(*args):
        operands = list(args)
        if partition_name is not None:
            operands.append(bass2jax.partition_id_tensor())
        outs = bass2jax._bass_exec_p.bind(
            *operands,
            out_avals=tuple(out_avals),
            in_names=tuple(all_names),
            out_names=tuple(out_names),
            lowering_input_output_aliases=(),
            sim_require_finite=True,
            sim_require_nnan=True,
            nc=nc,
        )
        return tuple(outs)

    devices = jax.devices()[:NCORES]
    mesh = Mesh(np.asarray(devices), ("core",))
    nin = n_params + len(out_names)
    sharded = jax.jit(
        shard_map(
            _body, mesh=mesh,
            in_specs=(PartitionSpec("core"),) * nin,
            out_specs=(PartitionSpec("core"),) * len(out_names),
            check_rep=False,
        ),
        keep_unused=True,
    )
    zeros_dev = None

    def run(in_maps):
        nonlocal zeros_dev
        concat_in = [
            np.concatenate([in_maps[c][name] for c in range(NCORES)], axis=0)
            for name in in_names
        ]
        if zeros_dev is None:
            zeros_dev = [jax.device_put(z) for z in zero_outs]
        out_arrs = sharded(*concat_in, *zeros_dev)
        full = [np.asarray(a) for a in out_arrs]
        return {
            name: full[i].reshape(NCORES, *out_avals[i].shape)
            for i, name in enumerate(out_names)
        }

    return run


def _host_prepare(inputs):
    ent = np.asarray(inputs["ent_emb"], dtype=np.float32)
    rel = np.asarray(inputs["rel_emb"], dtype=np.float32)
    attr = np.asarray(inputs["attr_emb"], dtype=np.float32)
    kern = np.asarray(inputs["attn_kernels"], dtype=np.float32)
    edge_index = np.asarray(inputs["edge_index"], dtype=np.int64)
    edge_rel = np.asarray(inputs["edge_rel"], dtype=np.int64)
    attr_index = np.asarray(inputs["attr_index"], dtype=np.int64)

    # rel table rows: [rel_emb(32) | srel2 per (l,h) (8) | pad(24)]
    rel_tab = np.zeros((R, 64), dtype=np.float32)
    rel_tab[:, 0:32] = rel
    for l in range(L):
        for h in range(H):
            rel_tab[:, 32 + l * 4 + h] = rel @ kern[l, h, 96:128]

    attr_tab = np.zeros((A, 64), dtype=np.float32)
    attr_tab[:, 0:32] = attr

    kv = np.zeros((L, 6, 128), dtype=np.float32)
    for l in range(L):
        kv[l, 0] = kern[l, :, 0:32].reshape(-1)
        kv[l, 1] = kern[l, :, 32:64].reshape(-1)
        kv[l, 2] = kern[l, :, 64:96].reshape(-1)
        kv[l, 3] = kern[l, :, 128:160].reshape(-1)
        kv[l, 4] = kern[l, :, 160:192].reshape(-1)
        kv[l, 5] = kern[l, :, 192:224].reshape(-1)
    kvecs = kv.reshape(1, -1)

    def pack(vals, deg, tail_pad=False):
        # vals: [NB, deg] int64 per core -> [TILES, 128, cols] int16
        padded = np.zeros((NPAD, deg), dtype=np.int64)
        padded[:NB] = vals
        v = padded.reshape(TILES, TP, deg).transpose(0, 2, 1).reshape(TILES, TP * deg)
        p16 = v.reshape(TILES, (TP * deg) // 16, 16).transpose(0, 2, 1)
        p16 = np.where(p16 < HALF, p16, p16 - 65536).astype(np.int16)
        if tail_pad:
            # one extra all-zero index column: the gather stream must not end
            # on a negative (sign-wrapped) index or the firmware drops the tail
            p16 = np.concatenate(
                [p16, np.zeros((TILES, 16, 1), np.int16)], axis=2)
        return np.tile(p16, (1, 8, 1))

    padmask = np.zeros((128, 1), dtype=np.float32)
    padmask[LASTP:, 0] = NEG_BIG

    cols = edge_index[:, 1].reshape(N, DEG)
    rels = edge_rel.reshape(N, DEG)
    aids = attr_index[:, 1].reshape(N, ADEG)

    in_maps = []
    for c in range(NCORES):
        lo, hi = c * NB, (c + 1) * NB
        in_maps.append({
            "ent_shard": np.ascontiguousarray(ent[lo:hi]),
            "rel_tab": rel_tab,
            "attr_tab": attr_tab,
            "kvecs": kvecs,
            "padmask": padmask,
            "cidx": pack(cols[lo:hi], DEG, tail_pad=True),
            "ridx": pack(rels[lo:hi], DEG),
            "aidx": pack(aids[lo:hi], ADEG),
        })
    return in_maps


def kernel(**inputs):
    if "run" not in _cached:
        nc = _build()
        nc.finalize() if not nc.is_finalized() else None
        _cached["run"] = _make_runner(nc)
    in_maps = _host_prepare(inputs)
    outs = _cached["run"](in_maps)
    return outs["out"].reshape(N, L * DOUT)


def kernel_debug(**inputs):
    if "ncd" not in _cached:
        _cached["ncd"] = _build(debug=True)
    nc = _cached["ncd"]
    in_maps = _host_prepare(inputs)
    res = run_bass_kernel_spmd(nc, in_maps, core_ids=list(range(NCORES)))
    out = np.concatenate([res.results[c]["out"] for c in range(NCORES)], axis=0)
    return out, [res.results[c]["dbg"] for c in range(NCORES)]


# revision 25
# speedup vs baseline: 71.0151x; 1.0038x over previous
"""GCAT (graph attention over ent/rel/attr embeddings) on 8 Trainium2 cores.

Sharding: edges are grouped 16-per-node and node-sorted, so we shard nodes
(and thus edges) into 8 contiguous blocks, one per core.  Embedding tables
and attention kernels are replicated.  Per layer each core computes its
nodes' features, all-gathers the [N,128] feature table (+ the [N,4]
per-node neighbor score table), then does per-edge gathers from the
replicated tables with dma_gather.

dma_gather indices are int16 (signed).  Rows >= 32768 are addressed via a
sign-wraparound trick: the gather base points at row 32768 of a region
whose first 17232 rows duplicate table rows 32768..49999, so negative
int16 indices (col - 65536) land on the duplicate copy.
"""

import sys

sys.path.insert(0, "/opt/trn_rl_repo")

import numpy as np

import concourse.bacc as bacc
import concourse.bass as bass
import concourse.bass_isa as bass_isa
import concourse.tile as tile
from concourse import mybir
from concourse.bass_utils import run_bass_kernel_spmd
from concourse.tile_rust import add_dep_helper

F32 = mybir.dt.float32
I16 = mybir.dt.int16
AF = mybir.ActivationFunctionType
OP = mybir.AluOpType

NCORES = 8
N = 50000
DEG = 16
E = N * DEG
R = 1000
A = 2000
ADEG = 4
DOUT = 128
H = 4
DH = 32
L = 2
NB = N // NCORES          # 6250 nodes per core
EB = NB * DEG             # 100000 edges per core
TP = 128                  # nodes per tile
TILES = (NB + TP - 1) // TP   # 49
NPAD = TILES * TP         # 6272
LASTP = NB - (TILES - 1) * TP  # 106 valid partitions in the last tile
HALF = 32768
EXTRA = N - HALF          # 17232 duplicated rows
RX = HALF + N             # 82768 region rows
NI = TP * DEG             # 2048 gather indices per tile
NI2 = NI + 16             # +16 zero-index pad so the stream never ends negative
                          # (the Q7 firmware drops a trailing run of negative
                          # int16 indices; mid-stream negatives are processed)
NIA = TP * ADEG           # 512 attr indices per tile
NEG_BIG = -1.0e9

_cached = {}


def _ap(t, width, offset, pairs, nparts=128, poff=0):
    """Strided view of a [nparts, width] SBUF tile. pairs = [[step, count], ...]"""
    base = t[:]
    return bass.AP(
        tensor=base.tensor,
        offset=base.offset + poff * width + offset,
        ap=[[width, nparts]] + [list(p) for p in pairs],
    )


def _build(debug=False):
    nc = bacc.Bacc(num_devices=NCORES)
    dbg = None
    if debug:
        dbg = nc.dram_tensor("dbg", [128, 704], F32, kind="ExternalOutput")

    ent_shard = nc.dram_tensor("ent_shard", [NB, DOUT], F32, kind="ExternalInput")
    rel_tab = nc.dram_tensor("rel_tab", [R, 64], F32, kind="ExternalInput")
    attr_tab = nc.dram_tensor("attr_tab", [A, 64], F32, kind="ExternalInput")
    kvecs = nc.dram_tensor("kvecs", [1, L * 6 * 128], F32, kind="ExternalInput")
    cidx = nc.dram_tensor("cidx", [TILES, 128, NI2 // 16], I16, kind="ExternalInput")
    ridx = nc.dram_tensor("ridx", [TILES, 128, NI // 16], I16, kind="ExternalInput")
    aidx = nc.dram_tensor("aidx", [TILES, 128, NIA // 16], I16, kind="ExternalInput")
    padmask = nc.dram_tensor("padmask", [128, 1], F32, kind="ExternalInput")
    out = nc.dram_tensor("out", [NB, L * DOUT], F32, kind="ExternalOutput")

    # Internal DRAM (per layer to avoid cross-layer races through collectives)
    region_x = [nc.dram_tensor(f"region_x{i}", [RX, DOUT], F32, kind="Internal",
                               addr_space="Shared") for i in range(L)]
    region_n = [nc.dram_tensor(f"region_n{i}", [RX, 64], F32, kind="Internal")
                for i in range(L)]
    n_full = [nc.dram_tensor(f"n_full{i}", [N, 4], F32, kind="Internal",
                             addr_space="Shared") for i in range(L)]
    stats_full = [nc.dram_tensor(f"stats_full{i}", [NCORES, 8], F32, kind="Internal",
                                 addr_space="Shared") for i in range(L)]
    region_e = nc.dram_tensor("region_e", [RX, DOUT], F32, kind="Internal",
                              addr_space="Shared")
    ent_bounce = nc.dram_tensor("ent_bounce", [NB, DOUT], F32, kind="Internal")
    agx_in = nc.dram_tensor("agx_in", [NB, DOUT], F32, kind="Internal")
    agn_in = nc.dram_tensor("agn_in", [NB, 4], F32, kind="Internal")
    stats_in = nc.dram_tensor("stats_in", [1, 8], F32, kind="Internal")

    RG = [list(range(NCORES))]

    with tile.TileContext(nc) as tc:
        with (
            tc.tile_pool(name="per", bufs=1) as per,          # persistent buffers
            tc.tile_pool(name="gbig", bufs=3) as gbig,        # 2048-col gather tiles
            tc.tile_pool(name="gsm", bufs=3) as gsm,          # 1024-col gather tiles
            tc.tile_pool(name="wgp", bufs=2) as wgp,          # weighted/tree scratch
            tc.tile_pool(name="idxp", bufs=2) as idxp,        # idx tiles
            tc.tile_pool(name="smp", bufs=2) as smp,          # small scratch
        ):
            W_E = TILES * 128      # 6272
            W_S = TILES * 64       # 3136
            W_C = TILES * 32       # 1568
            W_4 = TILES * 4        # 196

            e_all = per.tile([128, W_E], F32, tag="e_all")
            d_all = per.tile([128, W_E], F32, tag="d_all")
            srel_all = per.tile([128, W_E], F32, tag="srel_all")
            crel_all = per.tile([128, W_C], F32, tag="crel_all")
            cattr_all = per.tile([128, W_C], F32, tag="cattr_all")
            s_all = per.tile([128, W_S], F32, tag="s_all")
            z_all = per.tile([128, W_S], F32, tag="z_all")   # also: n_all, leaky scratch
            a_all = per.tile([128, W_S], F32, tag="a_all")
            self_all = per.tile([128, W_4], F32, tag="self_all")
            neigh_all = per.tile([128, W_4], F32, tag="neigh_all")
            t196 = per.tile([128, W_4], F32, tag="t196")
            mrow = per.tile([128, W_4], F32, tag="mrow")
            den = per.tile([128, W_4], F32, tag="den")
            kv_b = per.tile([128, 768], F32, tag="kv_b")
            m4a = per.tile([128, 4], F32, tag="m4a")   # mloc
            m4b = per.tile([128, 4], F32, tag="m4b")   # ssum
            m4c = per.tile([128, 4], F32, tag="m4c")   # B
            m4d = per.tile([128, 4], F32, tag="m4d")   # c = exp(mloc - B)
            m4e = per.tile([128, 4], F32, tag="m4e")   # scratch
            m4a2 = per.tile([128, 4], F32, tag="m4a2")
            m4b2 = per.tile([128, 4], F32, tag="m4b2")
            sb8c = per.tile([8, 4], F32, tag="sb8c")
            st8 = per.tile([8, 8], F32, tag="st8")
            sb8a = per.tile([8, 4], F32, tag="sb8a")
            sb8b = per.tile([8, 4], F32, tag="sb8b")
            stats_sb = per.tile([1, 8], F32, tag="stats_sb")
            pm = per.tile([128, 1], F32, tag="pm")
            nc.sync.dma_start(pm[:], padmask[:])

            def load_idx(src, t, cols):
                it = idxp.tile([128, 132], I16, tag="idx")
                nc.sync.dma_start(it[:, :cols], src[t, :, :])
                return it

            # build the ent gather region on device from the sharded input
            nc.sync.dma_start(ent_bounce[:], ent_shard[:])
            nc.gpsimd.collective_compute(
                "AllGather", OP.bypass, replica_groups=RG,
                ins=[ent_bounce[:]], outs=[region_e[HALF:, :]],
            )
            dupe = nc.sync.dma_start(region_e[0:EXTRA, :], region_e[2 * HALF:RX, :])

            # ---------------- phase 0: concepts + initial x ----------------
            for t in range(TILES):
                # relations: rows [rel_emb(32) | srel2(8) | pad]
                rit = load_idx(ridx, t, 128)
                grel = gsm.tile([128, 17 * 64], F32, tag="gsm")
                nc.gpsimd.dma_gather(
                    out_ap=grel[:, :NI // 128 * 64].rearrange("p (m d) -> p m d", d=64),
                    in_ap=rel_tab[:], idxs_ap=rit[:, :NI // 16],
                    num_idxs=NI, num_idxs_reg=NI, elem_size=64,
                    single_packet=False,
                )
                # crel = relu(sum_j rel[:, f] / 16): view (f, j) reduce X
                tcr = smp.tile([128, 32], F32, tag="c32")
                nc.vector.reduce_sum(
                    out=tcr[:],
                    in_=_ap(grel, 1088, 0, [[1, 32], [64, 16]]),
                    axis=mybir.AxisListType.X,
                )
                nc.scalar.activation(
                    out=crel_all[:, t * 32:(t + 1) * 32], in_=tcr[:],
                    func=AF.Relu, scale=1.0 / DEG,
                )
                # per-edge srel2 for both layers -> srel_all[(t,j,lh)]
                nc.vector.tensor_copy(
                    out=_ap(srel_all, W_E, t * 128, [[8, 16], [1, 8]]),
                    in_=_ap(grel, 1088, 32, [[64, 16], [1, 8]]),
                )

                # attributes
                ait = load_idx(aidx, t, 32)
                gat = gsm.tile([128, 256], F32, tag="gsma")
                nc.gpsimd.dma_gather(
                    out_ap=gat[:].rearrange("p (m d) -> p m d", d=64),
                    in_ap=attr_tab[:], idxs_ap=ait[:, :32],
                    num_idxs=NIA, num_idxs_reg=NIA, elem_size=64,
                    single_packet=False,
                )
                tca = smp.tile([128, 32], F32, tag="c32b")
                nc.vector.reduce_sum(
                    out=tca[:],
                    in_=_ap(gat, 256, 0, [[1, 32], [64, 4]]),
                    axis=mybir.AxisListType.X,
                )
                nc.scalar.activation(
                    out=cattr_all[:, t * 32:(t + 1) * 32], in_=tca[:],
                    func=AF.Relu, scale=1.0 / ADEG,
                )

                # initial x: mean of ent[col], then e0 = relu(x0)
                cit = load_idx(cidx, t, NI2 // 16)
                g = gbig.tile([128, 17 * DOUT], F32, tag="g")
                gei = nc.gpsimd.dma_gather(
                    out_ap=g[:].rearrange("p (m d) -> p m d", d=DOUT),
                    in_ap=region_e[HALF:, :], idxs_ap=cit[:, :NI2 // 16],
                    num_idxs=NI2, num_idxs_reg=NI2, elem_size=DOUT,
                    single_packet=False,
                )
                add_dep_helper(gei.ins, dupe.ins, sync=True,
                               reason="ent wraparound dup before gather")
                wg = wgp.tile([128, 2048], F32, tag="wg")
                nc.vector.tensor_add(wg[:, 0:1024], g[:, 0:1024], g[:, 1024:2048])
                nc.vector.tensor_add(wg[:, 0:512], wg[:, 0:512], wg[:, 512:1024])
                nc.vector.tensor_add(wg[:, 0:256], wg[:, 0:256], wg[:, 256:512])
                nc.vector.tensor_add(wg[:, 0:128], wg[:, 0:128], wg[:, 128:256])
                nc.scalar.activation(
                    out=e_all[:, t * 128:(t + 1) * 128], in_=wg[:, 0:128],
                    func=AF.Relu, scale=1.0 / DEG,
                )

            # ---------------- layers ----------------
            for l in range(L):
                kb = l * 6 * 128
                # broadcast-load this layer's kernel vectors [1,768] -> [128,768]
                nc.sync.dma_start(
                    kv_b[:],
                    bass.AP(tensor=kvecs[:].tensor, offset=kb,
                            ap=[[0, 128], [1, 768]]),
                )

                # --- per-node scores SELF/NEIGH via whole-buffer dots ---
                ev = _ap(e_all, W_E, 0, [[128, TILES], [32, 4], [1, 32]])
                dv = _ap(d_all, W_E, 0, [[128, TILES], [32, 4], [1, 32]])
                crv = _ap(crel_all, W_C, 0, [[32, TILES], [0, 4], [1, 32]])
                cav = _ap(cattr_all, W_C, 0, [[32, TILES], [0, 4], [1, 32]])

                def kvv(row):
                    return _ap(kv_b, 768, row * 128, [[0, TILES], [32, 4], [1, 32]])

                def dot_accum(dst, srcs):
                    # srcs: list of (in0_view, kv_row)
                    first = True
                    for in0, krow in srcs:
                        nc.vector.tensor_tensor(out=dv, in0=in0, in1=kvv(krow), op=OP.mult)
                        tgt = dst[:] if first else t196[:]
                        nc.vector.reduce_sum(
                            out=tgt,
                            in_=dv,
                            axis=mybir.AxisListType.X,
                        )
                        if not first:
                            nc.vector.tensor_add(dst[:], dst[:], t196[:])
                        first = False

                dot_accum(self_all, [(ev, 0), (crv, 1), (cav, 2)])
                dot_accum(neigh_all, [(crv, 3), (cav, 4), (ev, 5)])

                # --- write NEIGH and e to DRAM for the all-gathers ---
                nc.sync.dma_start(
                    bass.AP(tensor=agn_in[:].tensor, offset=0,
                            ap=[[4, 128], [TP * 4, TILES - 1], [1, 4]]),
                    _ap(neigh_all, W_4, 0, [[4, TILES - 1], [1, 4]]),
                )
                nc.sync.dma_start(
                    bass.AP(tensor=agn_in[:].tensor, offset=(TILES - 1) * TP * 4,
                            ap=[[4, LASTP], [1, 4]]),
                    _ap(neigh_all, W_4, (TILES - 1) * 4, [[1, 4]], nparts=LASTP),
                )
                nc.sync.dma_start(
                    bass.AP(tensor=agx_in[:].tensor, offset=0,
                            ap=[[128, 128], [TP * 128, TILES - 1], [1, 128]]),
                    _ap(e_all, W_E, 0, [[128, TILES - 1], [1, 128]]),
                )
                nc.sync.dma_start(
                    bass.AP(tensor=agx_in[:].tensor, offset=(TILES - 1) * TP * 128,
                            ap=[[128, LASTP], [1, 128]]),
                    _ap(e_all, W_E, (TILES - 1) * 128, [[1, 128]], nparts=LASTP),
                )

                nc.gpsimd.collective_compute(
                    "AllGather", OP.bypass, replica_groups=RG,
                    ins=[agn_in[:]], outs=[n_full[l][:]],
                )
                nc.gpsimd.collective_compute(
                    "AllGather", OP.bypass, replica_groups=RG,
                    ins=[agx_in[:]], outs=[region_x[l][HALF:, :]],
                )
                # expand neigh into padded region + duplicate blocks
                nc.sync.dma_start(region_n[l][HALF:, 0:4], n_full[l][:, :])
                dupn = nc.sync.dma_start(region_n[l][0:EXTRA, 0:4], n_full[l][HALF:N, :])
                dupx = nc.sync.dma_start(region_x[l][0:EXTRA, :], region_x[l][2 * HALF:RX, :])

                # --- pass 1: per-edge NEIGH gather -> scores ---
                for t in range(TILES):
                    cit = load_idx(cidx, t, NI2 // 16)
                    gn = gsm.tile([128, 17 * 64], F32, tag="gsm")
                    gni = nc.gpsimd.dma_gather(
                        out_ap=gn[:].rearrange("p (m d) -> p m d", d=64),
                        in_ap=region_n[l][HALF:, :], idxs_ap=cit[:, :NI2 // 16],
                        num_idxs=NI2, num_idxs_reg=NI2, elem_size=64,
                        single_packet=False,
                    )
                    # declared in_ap doesn't cover the wraparound duplicate rows
                    add_dep_helper(gni.ins, dupn.ins, sync=True,
                                   reason="neigh wraparound dup before gather")
                    nc.vector.tensor_copy(
                        out=_ap(z_all, W_S, t * 64, [[4, 16], [1, 4]]),
                        in_=_ap(gn, 1088, 0, [[64, 16], [1, 4]]),
                    )

                # s = n + srel[l] + self, then leaky_relu
                s_tjh = _ap(s_all, W_S, 0, [[64, TILES], [4, 16], [1, 4]])
                nc.vector.tensor_tensor(
                    out=s_tjh,
                    in0=_ap(z_all, W_S, 0, [[64, TILES], [4, 16], [1, 4]]),
                    in1=_ap(srel_all, W_E, l * 4, [[128, TILES], [8, 16], [1, 4]]),
                    op=OP.add,
                )
                nc.vector.tensor_tensor(
                    out=s_tjh, in0=s_tjh,
                    in1=_ap(self_all, W_4, 0, [[4, TILES], [0, 16], [1, 4]]),
                    op=OP.add,
                )
                nc.scalar.activation(out=z_all[:], in_=s_all[:], func=AF.Relu, scale=0.7)
                nc.vector.scalar_tensor_tensor(
                    out=s_all[:], in0=s_all[:], scalar=0.3, in1=z_all[:],
                    op0=OP.mult, op1=OP.add,
                )
                # mask pad rows of the last tile (additive -1e9 on pad partitions)
                lastc = (TILES - 1) * 64
                nc.vector.tensor_tensor(
                    out=s_all[:, lastc:lastc + 64],
                    in0=s_all[:, lastc:lastc + 64],
                    in1=pm[:, 0:1].to_broadcast([128, 64]),
                    op=OP.add,
                )

                # --- global softmax stats ---
                nc.vector.reduce_max(
                    out=m4a[:],
                    in_=_ap(s_all, W_S, 0, [[1, 4], [64, TILES], [4, 16]]),
                    axis=mybir.AxisListType.XY,
                )
                nc.gpsimd.partition_all_reduce(
                    m4a2[:], m4a[:], channels=128, reduce_op=bass_isa.ReduceOp.max)
                # z = exp(s - mloc)
                nc.vector.tensor_tensor(
                    out=_ap(z_all, W_S, 0, [[64, TILES], [4, 16], [1, 4]]),
                    in0=s_tjh,
                    in1=_ap(m4a2, 4, 0, [[0, TILES], [0, 16], [1, 4]]),
                    op=OP.subtract,
                )
                nc.scalar.activation(out=z_all[:], in_=z_all[:], func=AF.Exp)
                nc.vector.reduce_sum(
                    out=m4b[:],
                    in_=_ap(z_all, W_S, 0, [[1, 4], [64, TILES], [4, 16]]),
                    axis=mybir.AxisListType.XY,
                )
                nc.gpsimd.partition_all_reduce(
                    m4b2[:], m4b[:], channels=128, reduce_op=bass_isa.ReduceOp.add)

                # all-gather (mloc, ssum) and combine: B = M + ln(Z)
                nc.vector.tensor_copy(stats_sb[0:1, 0:4], m4a2[0:1, :])
                nc.vector.tensor_copy(stats_sb[0:1, 4:8], m4b2[0:1, :])
                nc.sync.dma_start(stats_in[:], stats_sb[:])
                nc.gpsimd.collective_compute(
                    "AllGather", OP.bypass, replica_groups=RG,
                    ins=[stats_in[:]], outs=[stats_full[l][:]],
                )
                nc.sync.dma_start(st8[:], stats_full[l][:])
                nc.gpsimd.partition_all_reduce(
                    sb8a[:], st8[:, 0:4], channels=8, reduce_op=bass_isa.ReduceOp.max)
                nc.vector.tensor_sub(sb8b[:], st8[:, 0:4], sb8a[:])
                nc.scalar.activation(out=sb8b[:], in_=sb8b[:], func=AF.Exp)
                nc.vector.tensor_mul(sb8b[:], sb8b[:], st8[:, 4:8])
                nc.gpsimd.partition_all_reduce(
                    sb8c[:], sb8b[:], channels=8, reduce_op=bass_isa.ReduceOp.add)
                nc.scalar.activation(out=sb8c[:], in_=sb8c[:], func=AF.Ln)
                nc.vector.tensor_add(sb8c[:], sb8c[:], sb8a[:])   # B per-partition(8)
                nc.gpsimd.partition_broadcast(m4c[:], sb8c[0:1, :])
                nc.vector.tensor_sub(m4e[:], m4a2[:], m4c[:])
                nc.scalar.activation(out=m4d[:], in_=m4e[:], func=AF.Exp)

                # s' = z * c ; per-row softmax -> a
                nc.vector.tensor_tensor(
                    out=s_tjh,
                    in0=_ap(z_all, W_S, 0, [[64, TILES], [4, 16], [1, 4]]),
                    in1=_ap(m4d, 4, 0, [[0, TILES], [0, 16], [1, 4]]),
                    op=OP.mult,
                )
                nc.vector.reduce_max(
                    out=mrow[:],
                    in_=_ap(s_all, W_S, 0, [[64, TILES], [1, 4], [4, 16]]),
                    axis=mybir.AxisListType.X,
                )
                nc.vector.tensor_tensor(
                    out=_ap(z_all, W_S, 0, [[64, TILES], [4, 16], [1, 4]]),
                    in0=s_tjh,
                    in1=_ap(mrow, W_4, 0, [[4, TILES], [0, 16], [1, 4]]),
                    op=OP.subtract,
                )
                nc.scalar.activation(out=z_all[:], in_=z_all[:], func=AF.Exp)
                nc.vector.reduce_sum(
                    out=den[:],
                    in_=_ap(z_all, W_S, 0, [[64, TILES], [1, 4], [4, 16]]),
                    axis=mybir.AxisListType.X,
                )
                nc.vector.reciprocal(out=den[:], in_=den[:])
                nc.vector.tensor_tensor(
                    out=_ap(a_all, W_S, 0, [[64, TILES], [4, 16], [1, 4]]),
                    in0=_ap(z_all, W_S, 0, [[64, TILES], [4, 16], [1, 4]]),
                    in1=_ap(den, W_4, 0, [[4, TILES], [0, 16], [1, 4]]),
                    op=OP.mult,
                )

                if debug and l == 0:
                    dt_ = smp.tile([128, 448], F32, tag="dbgt")
                    nc.vector.tensor_copy(dt_[:, 0:4], m4a2[:])
                    nc.vector.tensor_copy(dt_[:, 4:8], m4b2[:])
                    nc.vector.tensor_copy(dt_[:, 8:12], m4c[:])
                    nc.vector.tensor_copy(dt_[:, 12:16], m4d[:])
                    nc.vector.tensor_copy(dt_[:, 16:80], s_all[:, 0:64])      # s' tile0
                    nc.vector.tensor_copy(dt_[:, 80:144], a_all[:, 0:64])     # a tile0
                    nc.vector.tensor_copy(dt_[:, 144:176], crel_all[:, 0:32])
                    nc.vector.tensor_copy(dt_[:, 176:208], cattr_all[:, 0:32])
                    nc.vector.tensor_copy(dt_[:, 208:336], e_all[:, 0:128])   # e0 tile0
                    nc.vector.tensor_copy(dt_[:, 336:340], self_all[:, 0:4])
                    nc.vector.tensor_copy(dt_[:, 340:344], neigh_all[:, 0:4])
                    nc.vector.tensor_copy(dt_[:, 344:408], z_all[:, 0:64])
                    nc.vector.tensor_copy(dt_[:, 408:440], srel_all[:, 0:32])
                    nc.sync.dma_start(dbg[:, 0:448], dt_[:])

                # --- pass 2: gather e[col], weight, aggregate, tanh ---
                for t in range(TILES):
                    cit = load_idx(cidx, t, NI2 // 16)
                    g = gbig.tile([128, 17 * DOUT], F32, tag="g")
                    gxi = nc.gpsimd.dma_gather(
                        out_ap=g[:].rearrange("p (m d) -> p m d", d=DOUT),
                        in_ap=region_x[l][HALF:, :], idxs_ap=cit[:, :NI2 // 16],
                        num_idxs=NI2, num_idxs_reg=NI2, elem_size=DOUT,
                        single_packet=False,
                    )
                    add_dep_helper(gxi.ins, dupx.ins, sync=True,
                                   reason="x wraparound dup before gather")
                    wg = wgp.tile([128, 2048], F32, tag="wg")
                    for h in range(H):
                        nc.vector.tensor_tensor(
                            out=_ap(wg, 2048, h * 32, [[128, 16], [1, 32]]),
                            in0=_ap(g, 2176, h * 32, [[128, 16], [1, 32]]),
                            in1=_ap(a_all, W_S, t * 64 + h, [[4, 16], [0, 32]]),
                            op=OP.mult,
                        )
                    nc.vector.tensor_add(wg[:, 0:1024], wg[:, 0:1024], wg[:, 1024:2048])
                    nc.vector.tensor_add(wg[:, 0:512], wg[:, 0:512], wg[:, 512:1024])
                    nc.vector.tensor_add(wg[:, 0:256], wg[:, 0:256], wg[:, 256:512])
                    nc.vector.tensor_add(wg[:, 0:128], wg[:, 0:128], wg[:, 128:256])
                    xt = smp.tile([128, 128], F32, tag="xt")
                    nc.scalar.activation(out=xt[:], in_=wg[:, 0:128], func=AF.Tanh)
                    if debug and l == 0 and t == 0:
                        dt2 = smp.tile([128, 256], F32, tag="dbgt2")
                        nc.vector.tensor_copy(dt2[:, 0:128], g[:, 0:128])   # e_full[cols[p,0]]
                        nc.vector.tensor_copy(dt2[:, 128:256], xt[:])
                        nc.sync.dma_start(dbg[:, 448:704], dt2[:])
                    rp = TP if t < TILES - 1 else LASTP
                    nc.sync.dma_start(
                        bass.AP(tensor=out[:].tensor,
                                offset=t * TP * (L * DOUT) + l * DOUT,
                                ap=[[L * DOUT, rp], [1, DOUT]]),
                        xt[:rp, :],
                    )
                    if l + 1 < L:
                        nc.scalar.activation(
                            out=e_all[:, t * 128:(t + 1) * 128], in_=xt[:],
                            func=AF.Relu,
                        )

    nc.finalize()
    return nc




def _make_runner(nc):
    """Cached shard_map runner for the axon/PJRT path.

    Differences from bass_utils.run_bass_kernel_spmd: the jitted executable
    and the zero output templates are cached across calls, outputs are not
    donated (every element of "out" is written by the kernel), and each
    global output is fetched from device exactly once.
    """
    import jax
    from jax.sharding import Mesh, PartitionSpec
    from jax.experimental.shard_map import shard_map
    from concourse import bass2jax

    bass2jax.install_neuronx_cc_hook()
    partition_name = nc.partition_id_tensor.name if nc.partition_id_tensor else None
    in_names, out_names, out_avals, zero_outs = [], [], [], []
    for alloc in nc.m.functions[0].allocations:
        if not isinstance(alloc, mybir.MemoryLocationSet):
            continue
        name = alloc.memorylocations[0].name
        if alloc.kind == "ExternalInput":
            if name != partition_name:
                in_names.append(name)
        elif alloc.kind == "ExternalOutput":
            out_names.append(name)
            shape = tuple(alloc.tensor_shape)
            dtype = mybir.dt.np(alloc.dtype)
            out_avals.append(jax.core.ShapedArray(shape, dtype))
            zero_outs.append(np.zeros((NCORES * shape[0], *shape[1:]), dtype))
    n_params = len(in_names)
    all_names = list(in_names) + list(out_names)
    if partition_name is not None:
        all_names.append(partition_name)

    def _body(*args):
        operands = list(args)
        if partition_name is not None:
            operands.append(bass2jax.partition_id_tensor())
        outs = bass2jax._bass_exec_p.bind(
            *operands,
            out_avals=tuple(out_avals),
            in_names=tuple(all_names),
            out_names=tuple(out_names),
            lowering_input_output_aliases=(),
            sim_require_finite=True,
            sim_require_nnan=True,
            nc=nc,
        )
        return tuple(outs)

    devices = jax.devices()[:NCORES]
    mesh = Mesh(np.asarray(devices), ("core",))
    nin = n_params + len(out_names)
    sharded = jax.jit(
        shard_map(
            _body, mesh=mesh,
            in_specs=(PartitionSpec("core"),) * nin,
            out_specs=(PartitionSpec("core"),) * len(out_names),
            check_rep=False,
        ),
        keep_unused=True,
    )
    zeros_dev = None

    def run(in_maps):
        nonlocal zeros_dev
        concat_in = [
            np.concatenate([in_maps[c][name] for c in range(NCORES)], axis=0)
            for name in in_names
        ]
        if zeros_dev is None:
            zeros_dev = [jax.device_put(z) for z in zero_outs]
        out_arrs = sharded(*concat_in, *zeros_dev)
        full = [np.asarray(a) for a in out_arrs]
        return {
            name: full[i].reshape(NCORES, *out_avals[i].shape)
            for i, name in enumerate(out_names)
        }

    return run


def _host_prepare(inputs):
    ent = np.asarray(inputs["ent_emb"], dtype=np.float32)
    rel = np.asarray(inputs["rel_emb"], dtype=np.float32)
    attr = np.asarray(inputs["attr_emb"], dtype=np.float32)
    kern = np.asarray(inputs["attn_kernels"], dtype=np.float32)
    edge_index = np.asarray(inputs["edge_index"], dtype=np.int64)
    edge_rel = np.asarray(inputs["edge_rel"], dtype=np.int64)
    attr_index = np.asarray(inputs["attr_index"], dtype=np.int64)

    # rel table rows: [rel_emb(32) | srel2 per (l,h) (8) | pad(24)]
    rel_tab = np.zeros((R, 64), dtype=np.float32)
    rel_tab[:, 0:32] = rel
    for l in range(L):
        for h in range(H):
            rel_tab[:, 32 + l * 4 + h] = rel @ kern[l, h, 96:128]

    attr_tab = np.zeros((A, 64), dtype=np.float32)
    attr_tab[:, 0:32] = attr

    kv = np.zeros((L, 6, 128), dtype=np.float32)
    for l in range(L):
        kv[l, 0] = kern[l, :, 0:32].reshape(-1)
        kv[l, 1] = kern[l, :, 32:64].reshape(-1)
        kv[l, 2] = kern[l, :, 64:96].reshape(-1)
        kv[l, 3] = kern[l, :, 128:160].reshape(-1)
        kv[l, 4] = kern[l, :, 160:192].reshape(-1)
        kv[l, 5] = kern[l, :, 192:224].reshape(-1)
    kvecs = kv.reshape(1, -1)

    def pack(vals, deg, tail_pad=False):
        # vals: [NB, deg] int64 per core -> [TILES, 128, cols] int16
        padded = np.zeros((NPAD, deg), dtype=np.int64)
        padded[:NB] = vals
        v = padded.reshape(TILES, TP, deg).transpose(0, 2, 1).reshape(TILES, TP * deg)
        p16 = v.reshape(TILES, (TP * deg) // 16, 16).transpose(0, 2, 1)
        p16 = np.where(p16 < HALF, p16, p16 - 65536).astype(np.int16)
        if tail_pad:
            # one extra all-zero index column: the gather stream must not end
            # on a negative (sign-wrapped) index or the firmware drops the tail
            p16 = np.concatenate(
                [p16, np.zeros((TILES, 16, 1), np.int16)], axis=2)
        return np.tile(p16, (1, 8, 1))

    padmask = np.zeros((128, 1), dtype=np.float32)
    padmask[LASTP:, 0] = NEG_BIG

    cols = edge_index[:, 1].reshape(N, DEG)
    rels = edge_rel.reshape(N, DEG)
    aids = attr_index[:, 1].reshape(N, ADEG)

    in_maps = []
    for c in range(NCORES):
        lo, hi = c * NB, (c + 1) * NB
        in_maps.append({
            "ent_shard": np.ascontiguousarray(ent[lo:hi]),
            "rel_tab": rel_tab,
            "attr_tab": attr_tab,
            "kvecs": kvecs,
            "padmask": padmask,
            "cidx": pack(cols[lo:hi], DEG, tail_pad=True),
            "ridx": pack(rels[lo:hi], DEG),
            "aidx": pack(aids[lo:hi], ADEG),
        })
    return in_maps


def kernel(**inputs):
    if "run" not in _cached:
        nc = _build()
        nc.finalize() if not nc.is_finalized() else None
        _cached["run"] = _make_runner(nc)
    in_maps = _host_prepare(inputs)
    outs = _cached["run"](in_maps)
    return outs["out"].reshape(N, L * DOUT)


def kernel_debug(**inputs):
    if "ncd" not in _cached:
        _cached["ncd"] = _build(debug=True)
    nc = _cached["ncd"]
    in_maps = _host_prepare(inputs)
    res = run_bass_kernel_spmd(nc, in_maps, core_ids=list(range(NCORES)))
    out = np.concatenate([res.results[c]["out"] for c in range(NCORES)], axis=0)
    return out, [res.results[c]["dbg"] for c in range(NCORES)]
